# revision 23
# baseline (speedup 1.0000x reference)
"""Fused linear + cross-entropy (mean NLL) on 8 trn2 NeuronCores.

Strategy: vocab-parallel. Core c owns vocab rows [c*6656, (c+1)*6656) of
a 53248-padded vocab and computes, for ALL 8192 rows, the partial
softmax sums over its vocab shard.

Numerics: e ships as fp8e4 scaled x16; c ships as int4 codes (two per
byte, mid-rise quantizer with step = 0.34*std(c)), unpacked on device
by DVE (AND/SHR) + DVE/ACT casts into fp8 code values 0..15. Matmuls
run fp8 DoubleRow (K=256/instruction, 2x PE rate); the accumulator
holds 16*sum(e_hat*code), which the Exp activation rescales by
step/16. The int4 zero-offset (-7.5*step per c element) contributes a
per-row constant absorbed into lse on host via the quantized e rowsum.
The vocab bias is folded in as a 17th contraction chunk (values
bias/step, fp8) whose operands are built on device. Padded vocab rows
are code 0 with zero bias -> each contributes exp(0)=1, subtracted
exactly on host. The target logit e[n]·c[t_n] is a host-side f32
einsum. Host combines: lse = log(partials - n_pad) - 7.5*step*rowsum,
nll = lse - (tgt + bias[t]), mean over valid rows.

Transfers dominate wall time (the axon tunnel moves ~28 MB/s and
occasionally freezes whole-channel for ~40s before recovering), so the
kernel ships only ~146MB total (int4 c shards 109MB + fp8 e 34MB) with
device_puts issued asynchronously per core, overlapping the upload
with host prep, module build, and the walrus compile (AOT, in a
background thread); e is shipped as 8 disjoint shards and AllGathered
on device. The module is data-independent (fixed activation scale;
the measured quantizer step is absorbed into the e scaling) and BIR
debug info is stripped, so the NEFF and XLA caches hit across runs.
"""

import json
import time

import numpy as np
import ml_dtypes

import jax
import jax.numpy as jnp
from jax.sharding import Mesh, NamedSharding, PartitionSpec

import concourse.bass as bass
import concourse.tile as tile
from concourse import mybir
from concourse.bass_utils import run_bass_kernel_spmd
from concourse.tile import TileContext, ScopedClock

import os as _os

_NEFF_CACHE_DIR = _os.path.expanduser("~/.cache/bass_neff_ce")


def _install_neff_cache():
    """Memoize walrus compiles on disk keyed by BIR hash. The module is
    data-independent, so repeat runs skip the ~2.3s walrus subprocess."""
    import hashlib
    import shutil

    import concourse.bass2jax as b2j

    if getattr(b2j, "_neff_cache_installed", False):
        return
    b2j._neff_cache_installed = True
    orig = b2j.compile_bir_kernel

    def cached(ant_bir_str, compile_dir_path, neff_name="kernel.neff", **kw):
        try:
            _os.makedirs(_NEFF_CACHE_DIR, exist_ok=True)
            h = hashlib.sha256(
                ant_bir_str if isinstance(ant_bir_str, bytes) else ant_bir_str.encode()
            ).hexdigest()
            cpath = _os.path.join(_NEFF_CACHE_DIR, f"{h}.neff")
            out = _os.path.join(compile_dir_path, neff_name)
            if _os.path.exists(cpath):
                shutil.copyfile(cpath, out)
                return out
            res = orig(ant_bir_str, compile_dir_path, neff_name=neff_name, **kw)
            try:
                shutil.copyfile(res, cpath)
            except Exception:  # noqa: BLE001
                pass
            return res
        except Exception:  # noqa: BLE001
            return orig(ant_bir_str, compile_dir_path, neff_name=neff_name, **kw)

    b2j.compile_bir_kernel = cached


_cache_dir = _os.path.expanduser("~/.cache/jax_bass_ce")
try:
    _os.makedirs(_cache_dir, exist_ok=True)
    jax.config.update("jax_compilation_cache_dir", _cache_dir)
    jax.config.update("jax_persistent_cache_min_entry_size_bytes", -1)
    jax.config.update("jax_persistent_cache_min_compile_time_secs", 0.5)
except Exception:  # noqa: BLE001 - cache is best-effort
    pass

IGNORE_INDEX = -100

N, D, V = 8192, 4096, 50257
CORES = 8
VTILE = 512
VTC = 13                 # vocab tiles per core
VPC = VTC * VTILE        # vocab per core (6656)
VP = VPC * CORES         # padded vocab (53248)
NPAD = VP - V            # 2991 pad rows, all in core 7
KC2 = 16                 # double-K chunks (D = 16*256 exactly)
KPK = KC2 // 2           # packed chunk-pairs (8)
RT = N // 128            # 64 row tiles
RG = 8                   # row groups
RTG = RT // RG           # row tiles per group (8)
SCALE = 16.0             # nominal fp8 e scale at std(c)=0.02
ESH = 128 // CORES       # K-partition rows per e-shard (16)
STEP0 = 0.34 * 0.02      # nominal int4 step; the measured step/STEP0 ratio
                         # is absorbed into the fp8 e scaling so the module
                         # (and its BIR/NEFF) is data-independent

F8 = ml_dtypes.float8_e4m3

_PATCHED = False


def _patch_tile_drain():
    """This container's walrus rejects >1 sync-wait on a CTRL instruction;
    Tile's tail drain carries one wait per live semaphore. Split them into
    single-wait wait_ge ops on the sync queue."""
    global _PATCHED
    if _PATCHED:
        return
    _PATCHED = True

    import orjson

    # This walrus build accepts at most ONE sync-wait per instruction.
    # Post-process the serialized BIR: hoist extra waits onto same-engine
    # NoOps inserted directly before the instruction (identical sync
    # semantics - the engine stalls on each nop in turn).
    orig_to_json = bass.Bass.to_json_bytes

    def to_json_bytes_split(self, *a, **kw):
        m = orjson.loads(orig_to_json(self, *a, **kw))
        for f in m.get("functions", []):
            for blk in f.get("blocks", []):
                out = []
                for ins in blk["instructions"]:
                    # Drop source-location debug info: it embeds kernel.py
                    # line numbers, which would change the BIR (and defeat
                    # the NEFF/XLA caches) on every unrelated edit.
                    ins["debug"] = None
                    w = (ins.get("sync_info") or {}).get("on_wait") or []
                    if len(w) > 1:
                        for i, wi in enumerate(w[:-1]):
                            out.append(
                                {
                                    "debug": None,
                                    "engine": ins["engine"],
                                    "ins": [],
                                    "name": f"{ins['name']}-sw{i}",
                                    "opcode": "NoOp",
                                    "outs": [],
                                    "sync_info": {
                                        "on_update": [],
                                        "on_wait": [wi],
                                    },
                                }
                            )
                        ins["sync_info"]["on_wait"] = [w[-1]]
                    out.append(ins)
                blk["instructions"] = out
        return orjson.dumps(m)

    bass.Bass.to_json_bytes = to_json_bytes_split

    def _drain_and_barrier(self, tick_clock, wait_clock):
        nc = self.nc
        probe = nc.sync.nop(nofuse=True)
        wait_clock.add_sem_waits(
            probe.ins, ScopedClock({None: tick_clock.global_clock})
        )
        waits = list(probe.ins.sync_info.on_wait)
        probe.ins.sync_info.on_wait = []
        by_name = {h.name: h for h in self.sems.allocated().values()}
        for w in waits:
            nc.sync.wait_ge(by_name[w.ant_name], w.wait_value)
        nc.sync.drain()
        nc.all_engine_barrier()
        popped = nc._tile_sem_poison_stack.pop()
        assert popped is self._sem_poison
        nc.clear_and_free_semaphores(list(self.sems.allocated().values()))
        nc.all_engine_barrier()

    TileContext._drain_and_barrier = _drain_and_barrier


_NC_CACHE = {}


def _build_module():
    if "nc" in _NC_CACHE:
        return _NC_CACHE["nc"]
    _patch_tile_drain()
    f32 = mybir.dt.float32
    f8 = mybir.dt.float8e4
    u8 = mybir.dt.uint8
    bf = mybir.dt.bfloat16
    DR = mybir.MatmulPerfMode.DoubleRow
    AND = mybir.AluOpType.bitwise_and
    SHR = mybir.AluOpType.logical_shift_right
    MUL = mybir.AluOpType.mult

    nc = bass.Bass("TRN2", num_devices=CORES)
    # packed ct[vt, kp, kpk, two, vc]: low nibble = code of kc2=kpk, high
    # nibble = code of kc2=kpk+8, where
    # code[v, d] at d = kc2*256 + two*128 + kp, v = core_vocab + vt*512 + vc
    ct_d = nc.dram_tensor("ct", [VTC, 128, KPK, 2, VTILE], u8, kind="ExternalInput")
    # esh[kp_local, rt, kc2, two, m]: this core's 16-partition slab of the
    # full et[kp, rt, kc2, two, m] = e8[rt*128 + m, kc2*256 + two*128 + kp]
    esh_d = nc.dram_tensor("esh", [ESH, RT, KC2, 2, 128], f8, kind="ExternalInput")
    bias_d = nc.dram_tensor("biasv", [VTC, VTILE], f8, kind="ExternalInput")
    sume_d = nc.dram_tensor("sume", [128, RT, VTC], f32, kind="ExternalOutput")

    with TileContext(nc) as tc:
        with (
            tc.tile_pool(name="dram", bufs=1, space="DRAM") as dram,
            tc.tile_pool(name="singles", bufs=1) as singles,
            tc.tile_pool(name="etp", bufs=2) as etp,
            tc.tile_pool(name="ctpk", bufs=2) as ctpk,
            tc.tile_pool(name="ctu", bufs=2) as ctu,
            tc.tile_pool(name="ctp", bufs=2) as ctp,
            tc.tile_pool(name="ex", bufs=2) as exp_pool,
            tc.tile_pool(name="psum", bufs=6, space="PSUM") as psum,
        ):
            # --- AllGather e: each core ships 1/8 of et, gathers the rest ---
            e_bounce = dram.tile([ESH, RT, KC2, 2, 128], f8)
            et_full = dram.tile([128, RT, KC2, 2, 128], f8, addr_space="Shared")
            nc.gpsimd.dma_start(e_bounce[:], esh_d[:, :, :, :, :])
            nc.gpsimd.collective_compute(
                "AllGather",
                mybir.AluOpType.bypass,
                replica_groups=[list(range(CORES))],
                ins=[e_bounce.opt()],
                outs=[et_full.opt()],
            )

            # --- bias chunk operands, built on device ---
            # lhsT for the bias matmul: [128, 2, 128], only [0, 0, :] = SCALE
            e_bias = singles.tile([128, 2, 128], f8)
            nc.vector.memset(e_bias[:, :, :], 0)
            nc.vector.memset(e_bias[0:1, 0, :], SCALE)
            # rhs: [128, vt, 2, 512], only partition 0, two=0 holds bias/step
            bias_sb = singles.tile([128, VTC, 2, VTILE], f8)
            nc.vector.memset(bias_sb[:, :, :, :], 0)
            nc.sync.dma_start(out=bias_sb[0:1, :, 0, :], in_=bias_d[:, :])

            acc = singles.tile([128, RT, VTC], f32)

            for rg in range(RG):
                et_t = etp.tile([128, RTG, KC2, 2, 128], f8, tag="et")
                nc.sync.dma_start(
                    out=et_t, in_=et_full[:, rg * RTG : (rg + 1) * RTG]
                )
                for vt in range(VTC):
                    pk_t = ctpk.tile([128, KPK, 2, VTILE], u8, tag="pk")
                    nc.sync.dma_start(out=pk_t, in_=ct_d[vt])
                    lo_u = ctu.tile([128, KPK, 2, VTILE], u8, tag="lo")
                    hi_u = ctu.tile([128, KPK, 2, VTILE], u8, tag="hi")
                    nc.vector.tensor_scalar(
                        out=lo_u, in0=pk_t, scalar1=0x0F, scalar2=None, op0=AND
                    )
                    nc.vector.tensor_scalar(
                        out=hi_u, in0=pk_t, scalar1=4, scalar2=None, op0=SHR
                    )
                    ct_t = ctp.tile([128, KC2, 2, VTILE], f8, tag="ct")
                    nc.vector.tensor_scalar(
                        out=ct_t[:, 0:KPK], in0=lo_u, scalar1=1.0,
                        scalar2=None, op0=MUL,
                    )
                    nc.scalar.copy(out=ct_t[:, KPK:KC2], in_=hi_u)
                    for rt in range(RTG):
                        ps = psum.tile([128, VTILE], f32, tag="ps")
                        for k in range(KC2):
                            nc.tensor.matmul(
                                ps,
                                et_t[:, rt, k, :, :],
                                ct_t[:, k, :, :],
                                start=(k == 0),
                                stop=False,
                                perf_mode=DR,
                            )
                        nc.tensor.matmul(
                            ps,
                            e_bias[:, :, :],
                            bias_sb[:, vt, :, :],
                            start=False,
                            stop=True,
                            perf_mode=DR,
                        )
                        ex_t = exp_pool.tile([128, VTILE], bf, tag="ex")
                        nc.scalar.activation(
                            out=ex_t,
                            in_=ps,
                            func=mybir.ActivationFunctionType.Exp,
                            scale=STEP0 / SCALE,
                            accum_out=acc[:, rg * RTG + rt, vt : vt + 1],
                        )

            nc.gpsimd.dma_start(out=sume_d[:, :, :], in_=acc[:, :, :])

    _NC_CACHE["nc"] = nc
    return nc


def _quantize_c(c, inv_step):
    """int4 mid-rise codes with the two nibble planes pre-ORed (chunk
    pair d and d+2048): [V, D/2] u8. Elementwise only, so XLA compiles it
    fast and runs it multithreaded."""
    codes = jnp.clip(
        jnp.floor(c * inv_step) + 8.0, 0.0, 15.0
    ).astype(jnp.uint8)
    return codes[:, : D // 2] | (codes[:, D // 2 :] << 4)


def _tile_ct_shard(shard):
    """[VPC, D/2] packed u8 -> SBUF tile layout [VTC,128,KPK,2,VTILE]."""
    return shard.reshape(VTC, VTILE, KPK, 2, 128).transpose(0, 4, 2, 3, 1)


def _tgt_dot(e, c, safe_t):
    crows = jnp.take(c, safe_t, axis=0)
    return jnp.sum(e.astype(jnp.float32) * crows.astype(jnp.float32), axis=1)


def _prepare_exec(nc, mesh, timers):
    """Adapted from bass2jax.run_bass_via_pjrt: build + AOT-compile the
    sharded executable from abstract avals (no input data needed), so the
    walrus compile can run while the uploads stream."""
    from concourse.bass2jax import _bass_exec_p, partition_id_tensor

    partition_name = (
        nc.partition_id_tensor.name if nc.partition_id_tensor else None
    )
    in_names = []
    out_names = []
    in_avals = []
    out_avals = []
    for alloc in nc.m.functions[0].allocations:
        if not isinstance(alloc, mybir.MemoryLocationSet):
            continue
        name = alloc.memorylocations[0].name
        if alloc.kind == "ExternalInput":
            if name != partition_name:
                in_names.append(name)
                in_avals.append(
                    (tuple(alloc.tensor_shape), mybir.dt.np(alloc.dtype))
                )
        elif alloc.kind == "ExternalOutput":
            out_names.append(name)
            out_avals.append(
                jax.core.ShapedArray(
                    tuple(alloc.tensor_shape), mybir.dt.np(alloc.dtype)
                )
            )
    n_params = len(in_names)
    n_outs = len(out_avals)
    all_in_names = in_names + out_names
    if partition_name is not None:
        all_in_names.append(partition_name)

    def _body(*args):
        operands = list(args)
        if partition_name is not None:
            operands.append(partition_id_tensor())
        outs = _bass_exec_p.bind(
            *operands,
            out_avals=tuple(out_avals),
            in_names=tuple(all_in_names),
            out_names=tuple(out_names),
            lowering_input_output_aliases=(),
            sim_require_finite=True,
            sim_require_nnan=True,
            nc=nc,
        )
        return tuple(outs)

    donate = tuple(range(n_params, n_params + n_outs))
    from jax.experimental.shard_map import shard_map

    P = PartitionSpec
    fn = jax.jit(
        shard_map(
            _body,
            mesh=mesh,
            in_specs=(P("core"),) * (n_params + n_outs),
            out_specs=(P("core"),) * n_outs,
            check_rep=False,
        ),
        donate_argnums=donate,
        keep_unused=True,
    )
    sharding = NamedSharding(mesh, PartitionSpec("core"))
    arg_avals = [
        jax.ShapeDtypeStruct((CORES * s[0],) + tuple(s[1:]), d, sharding=sharding)
        for s, d in in_avals
    ] + [
        jax.ShapeDtypeStruct((CORES * a.shape[0],) + tuple(a.shape[1:]), a.dtype, sharding=sharding)
        for a in out_avals
    ]
    t0 = time.time()
    lowered = fn.lower(*arg_avals)
    timers["lower"] = time.time() - t0
    t0 = time.time()
    compiled = lowered.compile()
    timers["compile"] = time.time() - t0
    return compiled, in_names, out_names, out_avals


def _run_exec(compiled, in_names, out_names, out_avals, named_global, zeros_global, timers):
    inputs = [named_global[nm] for nm in in_names] + list(zeros_global)
    t0 = time.time()
    # The tunnel occasionally freezes whole-channel for ~40s and then
    # recovers on its own; re-issuing transfers only queues more bytes
    # behind the freeze, so just wait it out.
    args = [
        s.global_array if isinstance(s, _ShardedInput) else s for s in inputs
    ]
    for a in args:
        jax.block_until_ready(a)
    timers["input_wait"] = time.time() - t0
    t0 = time.time()
    outs = compiled(*args)
    jax.block_until_ready(outs)
    timers["run"] = time.time() - t0
    t0 = time.time()
    outs = [np.asarray(o) for o in outs]
    timers["fetch"] = time.time() - t0
    timers["exec"] = timers["input_wait"] + timers["run"] + timers["fetch"]
    return {
        name: outs[i].reshape(CORES, *out_avals[i].shape)
        for i, name in enumerate(out_names)
    }


class _ShardedInput:
    """Async per-core device_puts + the assembled global Array, keeping the
    host pieces so a stalled transfer can be cancelled and re-issued (the
    axon tunnel intermittently degrades in-flight streams to ~1MB/s)."""

    def __init__(self, pieces, devs, mesh):
        self.pieces = list(pieces)
        self.devs = devs
        self.sharding = NamedSharding(mesh, PartitionSpec("core"))
        shp = self.pieces[0].shape
        self.global_shape = (CORES * shp[0],) + tuple(shp[1:])
        self.bufs = [
            jax.device_put(self.pieces[i], devs[i]) for i in range(CORES)
        ]
        self._assemble()

    def _assemble(self):
        self.global_array = jax.make_array_from_single_device_arrays(
            self.global_shape, self.sharding, self.bufs
        )

    def ready_mask(self):
        mask = []
        for b in self.bufs:
            try:
                mask.append(bool(b.is_ready()))
            except Exception:  # noqa: BLE001
                mask.append(True)
        return mask

    def reput_unready(self):
        n = 0
        for i, ok in enumerate(self.ready_mask()):
            if not ok:
                try:
                    self.bufs[i].delete()
                except Exception:  # noqa: BLE001
                    pass
                self.bufs[i] = jax.device_put(self.pieces[i], self.devs[i])
                n += 1
        if n:
            self._assemble()
        return n


def _put_sharded(pieces, devs, mesh):
    return _ShardedInput(pieces, devs, mesh)


def _kernel_fast(e_np, c_np, bias_np, t_np, timers):
    import threading

    from concourse.bass2jax import install_neuronx_cc_hook

    install_neuronx_cc_hook()
    _install_neff_cache()
    devs = jax.devices()[:CORES]
    mesh = Mesh(np.asarray(devs), ("core",))
    cpu = jax.devices("cpu")[0]

    valid = t_np != IGNORE_INDEX
    safe_t = np.where(valid, t_np, 0)

    # step from a row-subsample of c: sampling error on std is ~0.1%,
    # irrelevant to the quantizer, and keeps the first upload early. The
    # measured/nominal ratio is folded into the e scaling so the module
    # keeps a fixed activation scale (deterministic BIR -> cacheable NEFF).
    t0 = time.time()
    step = 0.34 * float(c_np[::13].std())
    e_scale = SCALE * (step / STEP0)
    timers["std"] = time.time() - t0

    # --- module build + AOT walrus compile in a background thread; the
    # compile is a subprocess, so it overlaps host packing and uploads ---
    compile_box = {}

    def _builder():
        try:
            t0 = time.time()
            nc = _build_module()
            timers["build"] = time.time() - t0
            compile_box["ready"] = _prepare_exec(nc, mesh, timers)
        except Exception as err:  # noqa: BLE001
            compile_box["error"] = err

    th = threading.Thread(target=_builder, daemon=True)
    th.start()

    # --- tiny inputs first, then the big ct shards, biggest-first so the
    # wire is saturated as early as possible ---
    t0 = time.time()
    b8 = np.zeros((VP,), dtype=F8)
    b8[:V] = (bias_np / step).astype(F8)
    bias_global = _put_sharded(b8.reshape(CORES, VTC, VTILE), devs, mesh)
    zeros_global = [
        _put_sharded(np.zeros((CORES, 128, RT, VTC), np.float32), devs, mesh)
    ]
    timers["prep_small"] = time.time() - t0

    t0 = time.time()
    with jax.default_device(cpu):
        packed_vd = np.asarray(jax.jit(_quantize_c)(c_np, 1.0 / step))
    timers["quant_c"] = time.time() - t0
    t0 = time.time()
    ct_pieces = []
    with jax.default_device(cpu):
        tile_ct = jax.jit(_tile_ct_shard)
        for cid in range(CORES):
            lo = cid * VPC
            hi = min((cid + 1) * VPC, V)
            if hi - lo == VPC:
                shard = packed_vd[lo:hi]
            else:
                shard = np.zeros((VPC, D // 2), np.uint8)
                shard[: hi - lo] = packed_vd[lo:hi]
            ct_pieces.append(np.asarray(tile_ct(shard)))
    ct_global = _ShardedInput(ct_pieces, devs, mesh)
    timers["prep_c"] = time.time() - t0

    # --- e: fp8 cast + tile transpose + quantized rowsum (numpy) ---
    t0 = time.time()
    e8_flat = (e_np * e_scale).astype(F8)
    et = np.ascontiguousarray(
        e8_flat.reshape(RT, 128, KC2, 2, 128).transpose(4, 0, 2, 3, 1)
    )
    esh = et.reshape(CORES, ESH, RT, KC2, 2, 128)
    esh_global = _put_sharded(esh, devs, mesh)
    timers["prep_e"] = time.time() - t0

    # --- overlapped with the uploads: host math ---
    t0 = time.time()
    rowsum_q = (
        e8_flat.astype(np.float32).sum(axis=1, dtype=np.float64) / e_scale
    )
    with jax.default_device(cpu):
        tgt_host = np.asarray(
            jax.jit(_tgt_dot)(e_np, c_np, safe_t)
        ).astype(np.float64)
    timers["host_math"] = time.time() - t0

    t0 = time.time()
    th.join()
    timers["compile_wait"] = time.time() - t0
    if "error" in compile_box:
        raise compile_box["error"]
    compiled, in_names, out_names, out_avals = compile_box["ready"]

    res = _run_exec(
        compiled,
        in_names,
        out_names,
        out_avals,
        {"ct": ct_global, "esh": esh_global, "biasv": bias_global},
        zeros_global,
        timers,
    )

    # --- combine ---
    t0 = time.time()
    S = res["sume"].astype(np.float64).sum(axis=(0, 3))  # [128, RT]
    S = S.T.reshape(N) - NPAD  # row n = rt*128 + p; pads contribute exp(0)=1
    lse = np.log(S) - 7.5 * step * rowsum_q
    tgt_logit = tgt_host + bias_np.astype(np.float64)[safe_t]
    nll = np.where(valid, lse - tgt_logit, 0.0)
    n_valid = max(int(valid.sum()), 1)
    timers["combine"] = time.time() - t0
    return np.float32(nll.sum() / n_valid)


def _kernel_fallback(e_np, c_np, bias_np, t_np, timers):
    """Safety net: same math, stock run_bass_kernel_spmd path."""
    valid = t_np != IGNORE_INDEX
    safe_t = np.where(valid, t_np, 0)
    c_f32 = np.asarray(c_np, np.float32)
    step = 0.34 * float(c_f32.std())
    e_scale = SCALE * (step / STEP0)
    inv = 1.0 / step
    codes = np.clip(np.floor(c_f32 * inv) + 8.0, 0.0, 15.0).astype(np.uint8)
    codes = np.concatenate(
        [codes, np.zeros((VP - V, D), np.uint8)], axis=0
    ).reshape(CORES, VTC, VTILE, KC2, 2, 128).transpose(0, 1, 5, 3, 4, 2)
    ct_packed = codes[:, :, :, 0:KPK] | (codes[:, :, :, KPK:KC2] << 4)

    e8_flat = (e_np * e_scale).astype(F8)
    et = np.ascontiguousarray(
        e8_flat.reshape(RT, 128, KC2, 2, 128).transpose(4, 0, 2, 3, 1)
    )
    esh = et.reshape(CORES, ESH, RT, KC2, 2, 128)
    b8 = np.zeros((VP,), dtype=F8)
    b8[:V] = (bias_np / step).astype(F8)
    bt = b8.reshape(CORES, VTC, VTILE)

    rowsum_q = (
        e8_flat.astype(np.float32).sum(axis=1, dtype=np.float64) / e_scale
    )
    tgt_host = np.einsum("nd,nd->n", e_np, c_f32[safe_t]).astype(np.float64)

    nc = _build_module()
    in_maps = [
        {"ct": ct_packed[i], "esh": esh[i], "biasv": bt[i]} for i in range(CORES)
    ]
    t0 = time.time()
    res = run_bass_kernel_spmd(nc, in_maps, core_ids=list(range(CORES)))
    timers["exec"] = time.time() - t0
    S = np.stack([r["sume"] for r in res.results]).astype(np.float64)
    S = S.sum(axis=(0, 3)).T.reshape(N) - NPAD
    lse = np.log(S) - 7.5 * step * rowsum_q
    tgt_logit = tgt_host + bias_np.astype(np.float64)[safe_t]
    nll = np.where(valid, lse - tgt_logit, 0.0)
    n_valid = max(int(valid.sum()), 1)
    return np.float32(nll.sum() / n_valid)


def kernel(e, c, bias, targets, _trace=False):
    timers = {}
    kernel.timers = timers
    t_all = time.time()
    e_np = np.asarray(e, dtype=np.float32)
    c_np = np.asarray(c, dtype=np.float32)
    bias_np = np.asarray(bias, dtype=np.float32)
    t_np = np.asarray(targets).astype(np.int64)
    try:
        out = _kernel_fast(e_np, c_np, bias_np, t_np, timers)
    except Exception as err:  # pragma: no cover - safety net
        import traceback

        traceback.print_exc()
        print(f"fast path failed ({err!r}); falling back", flush=True)
        out = _kernel_fallback(e_np, c_np, bias_np, t_np, timers)
    timers["total"] = time.time() - t_all
    kernel.last_run_wall_s = timers.get("exec", timers["total"])
    return out
